# revision 15
# baseline (speedup 1.0000x reference)
"""Multi-head dot-product attention on 8 Trainium2 NeuronCores.

Sharding: 8 cores = 4 batches x 2 head-groups (8 heads each).
Each core computes its batch's QKV projections (its 8 heads), full
softmax attention for those heads, and a partial output projection.
The host sums the two head-group partials per batch and adds the
(linear) bo / bv contributions.

Per-core pipeline (all matmuls bf16 with fp32 PSUM accumulation):
  A: Q^T,K^T [hd, L] and V [L, hd] projections from host-transposed X^T
  B: head-PAIR structure, 512-wide q chunks. All phase-B matmuls run in
     64x128 row-tiled mode: the pair's scores S^T execute concurrently on
     PE row tiles T0/T8 (head 2j on SBUF partitions 0-63, head 2j+1 on
     64-127), and each P.V matmul is split into two sequential 64-row
     half-contractions accumulating into one PSUM tile, so the PE never
     pays a tiling-mode-switch drain inside the loop.
     Exp evacuation of S^T (PSUM fp32 -> SBUF bf16 P) is split between
     ScalarE (exact exp LUT) and VectorE (Schraudolph bit-trick exp:
     one tensor_scalar mult+add with int16 round-to-nearest output whose
     bits are the bf16 of 2^(s*log2e); ~1.8% rms multiplicative noise on
     ~40% of tiles, gate impact ~9e-3) so neither engine bottlenecks.
     A ones-column appended to V yields softmax denominators in PSUM row
     64 for free; denominators are reciprocated straight out of PSUM via
     the single-pass DVE reciprocal_approx_fast, staged to DRAM, and
     partition-broadcast back; the normalization multiplies are deferred
     one qc-block so the DMA roundtrip never stalls the VectorE queue.
  C: output projection (partial - contracts this core's 8 heads)

Shapes (hardcoded): B=4, L=2048, D=1024, H=16, Hd=64.
"""

import os
import sys

for _p in ("/opt/trn_rl_repo", os.path.expanduser("~/.axon_site/_ro/trn_rl_repo")):
    if os.path.isdir(_p) and _p not in sys.path:
        sys.path.insert(0, _p)

from contextlib import ExitStack

import ml_dtypes
import numpy as np

import concourse.bass as bass
import concourse.tile as tile
from concourse import bacc, mybir
from concourse.bass_utils import run_bass_kernel_spmd

F32 = mybir.dt.float32
BF16 = mybir.dt.bfloat16
I16 = mybir.dt.int16

B, L, D, H, Hd = 4, 2048, 1024, 16, 64
HG = H // 2  # heads per core (head group)
HDG = HG * Hd  # 512: per-core projected width
KT = L // 128  # 16 k/l tiles
MT = HDG // 128  # 4 hd tiles
NQC = 4  # 512-wide q chunks
QW = L // NQC  # 512
EXP_SCALE = 1.0 / np.sqrt(Hd)
# Schraudolph exp: bf16 bits = int16(s * EXP_C1 + EXP_C0)
EXP_C1 = float(128.0 * np.log2(np.e) * EXP_SCALE)
EXP_C0 = 16248.5
# per-t: head a always evacuates via ScalarE exp; head b via VectorE
# Schraudolph except these t's (ACT:DVE = 19:13 per pair-unit)
ACT_EXTRA_T = frozenset({2, 7, 12})

# HW-validated configuration (hw_bisect.py): PV half-accumulation across
# PE row tiles fails at runtime, and reciprocal_approx_fast reading PSUM
# directly returns garbage — keep PV as full-128 matmuls and stage the
# denominator row through SBUF before the reciprocal.
T_SCHRAU = os.environ.get("T_SCHRAU", "1") == "1"  # DVE Schraudolph exp
T_PVHALF = os.environ.get("T_PVHALF", "0") == "1"  # PV 64-row half-accum
T_RECIP = os.environ.get("T_RECIP", "0") == "1"    # recip direct from PSUM


def build_program(loop_n=1, phases='ABC'):
    nc = bacc.Bacc()

    xt_d = nc.dram_tensor("xt", [D, L], BF16, kind="ExternalInput")
    wq_d = nc.dram_tensor("wq", [D, HDG], BF16, kind="ExternalInput")
    wk_d = nc.dram_tensor("wk", [D, HDG], BF16, kind="ExternalInput")
    wv_d = nc.dram_tensor("wv", [D, HDG], BF16, kind="ExternalInput")
    wo_d = nc.dram_tensor("wo", [HDG, D], BF16, kind="ExternalInput")
    bq_d = nc.dram_tensor("bq", [HDG], F32, kind="ExternalInput")
    bk_d = nc.dram_tensor("bk", [HDG], F32, kind="ExternalInput")
    onesv_d = nc.dram_tensor("onesv", [128, KT, HG], BF16, kind="ExternalInput")
    recip_d = nc.dram_tensor("recip_scratch", [HG * NQC, QW], F32)
    y_d = nc.dram_tensor("y", [L, D], F32, kind="ExternalOutput")

    with tile.TileContext(nc) as tc, \
            nc.allow_low_precision(reason="bf16/approx-exp attention internals are intentional"):
        if loop_n == 1:
            with ExitStack() as ctx:
                kernel_body(ctx, tc, xt_d, wq_d, wk_d, wv_d, wo_d, bq_d, bk_d,
                            onesv_d, recip_d, y_d, phases)
        else:
            with tc.For_i(0, loop_n, 1):
                with ExitStack() as ctx:
                    kernel_body(ctx, tc, xt_d, wq_d, wk_d, wv_d, wo_d, bq_d,
                                bk_d, onesv_d, recip_d, y_d, phases)
    nc.compile()
    return nc


def kernel_body(ctx, tc, xt_d, wq_d, wk_d, wv_d, wo_d, bq_d, bk_d,
                onesv_d, recip_d, y_d, phases="ABC"):
    nc = tc.nc
    Exp = mybir.ActivationFunctionType.Exp

    persist = ctx.enter_context(tc.tile_pool(name="persist", bufs=1))

    # persistent tensors
    qt_sb = persist.tile([128, MT, L], BF16)   # Q^T: [hd-in-tile, m, l]
    kt_sb = persist.tile([128, MT, L], BF16)   # K^T
    v_sb = persist.tile([128, KT, HG * (Hd + 1)], BF16)  # V + ones col per head
    bq_sb = persist.tile([128, MT], F32)
    bk_sb = persist.tile([128, MT], F32)

    nc.sync.dma_start(bq_sb[:], bq_d.ap().rearrange("(m p) -> p m", p=128))
    nc.sync.dma_start(bk_sb[:], bk_d.ap().rearrange("(m p) -> p m", p=128))
    # ones columns of V (col Hd of each head's 65-wide block)
    vv = v_sb[:].rearrange("p t (h c) -> p t h c", h=HG)
    nc.sync.dma_start(vv[:, :, :, Hd : Hd + 1].rearrange("p t h c -> p t (h c)"),
                      onesv_d.ap())

    # ---------------- Phase A: QKV projections ----------------
    with tc.tile_pool(name="xtp", bufs=1) as xtp, \
         tc.tile_pool(name="wvp", bufs=1) as wvp, \
         tc.tile_pool(name="wst", bufs=6) as wst, \
         tc.tile_pool(name="ppa", bufs=8, space="PSUM") as ppa:
        xt_sb = xtp.tile([128, D // 128, L], BF16)
        xt_r = xt_d.ap().rearrange("(k p) n -> p k n", p=128)
        dmae = [nc.sync, nc.gpsimd]
        for k in range(D // 128):
            dmae[k % 2].dma_start(xt_sb[:, k, :], xt_r[:, k, :])

        # Q^T, K^T: out[m-tile] = W[:, m-tile].T @ X^T, chunks of 512 over l.
        # m-tile outer / tensor inner so kt[0] (which gates phase B's first
        # scores) is ready after the first m pass, not after all of Q.
        wq_r = wq_d.ap().rearrange("(k p) m -> p k m", p=128)
        wk_r = wk_d.ap().rearrange("(k p) m -> p k m", p=128)
        for m in range(MT):
            for w_r, dst_sb, bias_sb in ((wq_r, qt_sb, bq_sb),
                                         (wk_r, kt_sb, bk_sb)):
                psums = []
                for lc in range(4):
                    psums.append(ppa.tile([128, 512], F32, tag="pp", name=f"pp{lc}"))
                wt = wst.tile([128, D // 128, 128], BF16, tag="wt")
                dmae[m % 2].dma_start(wt[:], w_r[:, :, m * 128 : (m + 1) * 128])
                for k in range(D // 128):
                    for lc in range(4):
                        nc.tensor.matmul(
                            psums[lc][:],
                            wt[:, k, :],
                            xt_sb[:, k, lc * 512 : (lc + 1) * 512],
                            start=(k == 0),
                            stop=(k == D // 128 - 1),
                        )
                for lc in range(4):
                    nc.scalar.activation(
                        dst_sb[:, m, lc * 512 : (lc + 1) * 512],
                        psums[lc][:],
                        mybir.ActivationFunctionType.Identity,
                        bias=bias_sb[:, m : m + 1],
                    )

        # V in natural layout: V[l-tile] = X^T[:, l-tile].T @ Wv
        wv_sb = wvp.tile([128, D // 128, HDG], BF16)
        wv_r = wv_d.ap().rearrange("(k p) n -> p k n", p=128)
        for k in range(D // 128):
            dmae[k % 2].dma_start(wv_sb[:, k, :], wv_r[:, k, :])
        for lt in range(KT):
            ps_v = ppa.tile([128, 512], F32, tag="pp")
            for k in range(D // 128):
                nc.tensor.matmul(
                    ps_v[:],
                    xt_sb[:, k, lt * 128 : (lt + 1) * 128],
                    wv_sb[:, k, :],
                    start=(k == 0),
                    stop=(k == D // 128 - 1),
                )
            nc.vector.tensor_copy(
                vv[:, lt, :, 0:Hd],
                ps_v[:].rearrange("p (h c) -> p h c", h=HG),
            )

    # ---------------- Phase B: attention ----------------
    if "B" not in phases:
        return
    with tc.tile_pool(name="otn", bufs=1) as otnp:
      otn_sb = otnp.tile([128, MT, L], BF16)  # normalized O^T
      with tc.tile_pool(name="pb", bufs=8) as pb, \
           tc.tile_pool(name="nrm", bufs=6) as nrm, \
           tc.tile_pool(name="psb", bufs=4, space="PSUM") as psb, \
           tc.tile_pool(name="pob", bufs=2, space="PSUM") as pob:

        # deferred normalization closures, flushed one qc-block later
        pending_norm = []

        def flush_norm():
            while pending_norm:
                pending_norm.pop(0)()

        def make_norm(j, qc):
            # broadcast DMAs issue NOW (the recip rows are already on their
            # way to DRAM); only the multiply is deferred one qc block
            q_sl = slice(qc * QW, (qc + 1) * QW)
            rb = nrm.tile([128, QW], F32, tag="rb")
            for hi in range(2):
                row = qc * HG + 2 * j + hi
                nc.gpsimd.dma_start(
                    rb[hi * 64 : hi * 64 + 64, :],
                    recip_d.ap()[row : row + 1, :].partition_broadcast(64))

            def run():
                sl = otn_sb[:, j, q_sl]
                nc.vector.tensor_tensor(sl, sl, rb[:],
                                        op=mybir.AluOpType.mult)
            return run

        for qc in range(NQC):
            q_sl = slice(qc * QW, (qc + 1) * QW)
            for j in range(MT):
                heads = (2 * j, 2 * j + 1)
                po = [pob.tile([65, QW], F32, tag=f"po{hi}", name=f"po{hi}")
                      for hi in range(2)]
                prev = [None, None]
                for t in range(KT + 1):
                    cur = [None, None]
                    if t < KT:
                        ps_s = [None, None]
                        # pair scores, concurrent on PE row tiles T0/T8
                        for hi in range(2):
                            r = hi * 64
                            ps_s[hi] = psb.tile([128, QW], F32, tag="ps",
                                                name=f"ps{hi}")
                            nc.tensor.matmul(
                                ps_s[hi][:],
                                kt_sb[r : r + 64, j, t * 128 : (t + 1) * 128],
                                qt_sb[r : r + 64, j, q_sl],
                                start=True,
                                stop=True,
                            )
                        for hi in range(2):
                            pt = pb.tile([128, QW], BF16, tag="pt", name="pt")
                            if not T_SCHRAU or hi == 0 or t in ACT_EXTRA_T:
                                nc.scalar.activation(pt[:], ps_s[hi][:], Exp,
                                                     scale=EXP_SCALE)
                            else:
                                nc.vector.tensor_scalar(
                                    pt[:].bitcast(I16), ps_s[hi][:],
                                    EXP_C1, EXP_C0,
                                    op0=mybir.AluOpType.mult,
                                    op1=mybir.AluOpType.add,
                                )
                            cur[hi] = pt
                    if t > 0:
                        # P.V as two sequential 64-row half-contractions
                        # accumulating into one PSUM tile; issue order
                        # (a0, b1), (a1, b0) keeps both PE row tiles busy
                        # without a same-bank conflict.
                        h0, h1 = heads
                        c0 = h0 * (Hd + 1)
                        c1 = h1 * (Hd + 1)
                        if T_PVHALF:
                            # (hi, half, first_issue, last_issue): start/stop
                            # must sit on each head's first/last ISSUED matmul
                            for hi, half, first, last in (
                                    (0, 0, True, False), (1, 1, True, False),
                                    (0, 1, False, True), (1, 0, False, True)):
                                r = half * 64
                                cc = c0 if hi == 0 else c1
                                nc.tensor.matmul(
                                    po[hi][:],
                                    v_sb[r : r + 64, t - 1, cc : cc + Hd + 1],
                                    prev[hi][r : r + 64, :],
                                    start=(t == 1 and first),
                                    stop=(t == KT and last),
                                )
                        else:
                            for hi in range(2):
                                cc = c0 if hi == 0 else c1
                                nc.tensor.matmul(
                                    po[hi][:],
                                    v_sb[:, t - 1, cc : cc + Hd + 1],
                                    prev[hi][:],
                                    start=(t == 1),
                                    stop=(t == KT),
                                )
                    prev = cur
                # unit end: evacuate unnormalized O^T rows + reciprocals
                for hi in range(2):
                    h = heads[hi]
                    r = hi * 64
                    row = qc * HG + h
                    nc.vector.tensor_copy(otn_sb[r : r + 64, j, q_sl],
                                          po[hi][0:64, :])
                    # DVE single-partition writes must start at partition 0:
                    # stage the reciprocal row in a rotating tile, then DMA
                    rrow = nrm.tile([128, QW], F32, tag="rr", bufs=3)
                    if T_RECIP:
                        nc.vector.reciprocal_approx_fast(
                            rrow[0:1, :], po[hi][64:65, :])
                    else:
                        rr2 = nrm.tile([128, QW], F32, tag="rr2", bufs=3)
                        nc.vector.tensor_copy(rr2[0:1, :], po[hi][64:65, :])
                        nc.vector.reciprocal_approx_fast(
                            rrow[0:1, :], rr2[0:1, :])
                    nc.sync.dma_start(recip_d.ap()[row : row + 1, :],
                                      rrow[0:1, :])
                # run one deferred norm from the previous qc block BEFORE
                # allocating this unit's rb tile (so a freed rb slot's last
                # reader is always emitted before its next writer); by now
                # its recip DMA roundtrip has long completed, so the DVE
                # queue never stalls on it
                while len(pending_norm) >= MT + 1:
                    pending_norm.pop(0)()
                pending_norm.append(make_norm(j, qc))
        flush_norm()

      # ---------------- Phase C: output projection ----------------
      if "C" not in phases:
          return
      with tc.tile_pool(name="wop", bufs=1) as wop, \
           tc.tile_pool(name="yst", bufs=4) as yst, \
           tc.tile_pool(name="pyc", bufs=4, space="PSUM") as pyc:
          wo_sb = wop.tile([128, MT, D], BF16)
          nc.sync.dma_start(wo_sb[:], wo_d.ap().rearrange("(k p) n -> p k n", p=128))
          for mq in range(KT):
              for nch in range(2):
                  n_sl = slice(nch * 512, (nch + 1) * 512)
                  ps_y = pyc.tile([128, 512], F32, tag="py")
                  for k in range(MT):
                      nc.tensor.matmul(
                          ps_y[:],
                          otn_sb[:, k, mq * 128 : (mq + 1) * 128],
                          wo_sb[:, k, n_sl],
                          start=(k == 0),
                          stop=(k == MT - 1),
                      )
                  yt = yst.tile([128, 512], F32, tag="yt")
                  if (mq + nch) % 2 == 0:
                      nc.vector.tensor_copy(yt[:], ps_y[:])
                  else:
                      nc.scalar.activation(
                          yt[:], ps_y[:],
                          mybir.ActivationFunctionType.Identity)
                  nc.sync.dma_start(
                      y_d.ap()[mq * 128 : (mq + 1) * 128, n_sl], yt[:])


_PROGRAM_CACHE = {}


def _get_program():
    if "nc" not in _PROGRAM_CACHE:
        _PROGRAM_CACHE["nc"] = build_program()
    return _PROGRAM_CACHE["nc"]


def make_in_maps(inputs):
    x = np.asarray(inputs["x"], dtype=np.float32)
    wq = np.asarray(inputs["wq"], dtype=np.float32)
    wk = np.asarray(inputs["wk"], dtype=np.float32)
    wv = np.asarray(inputs["wv"], dtype=np.float32)
    wo = np.asarray(inputs["wo"], dtype=np.float32)
    bq = np.asarray(inputs["bq"], dtype=np.float32)
    bk = np.asarray(inputs["bk"], dtype=np.float32)

    onesv = np.ones((128, KT, HG), dtype=ml_dtypes.bfloat16)

    in_maps = []
    for c in range(8):
        b, g = divmod(c, 2)
        hs = slice(g * HG, (g + 1) * HG)
        in_maps.append({
            "xt": np.ascontiguousarray(x[b].T).astype(ml_dtypes.bfloat16),
            "wq": np.ascontiguousarray(wq[:, hs, :]).reshape(D, HDG).astype(ml_dtypes.bfloat16),
            "wk": np.ascontiguousarray(wk[:, hs, :]).reshape(D, HDG).astype(ml_dtypes.bfloat16),
            "wv": np.ascontiguousarray(wv[:, hs, :]).reshape(D, HDG).astype(ml_dtypes.bfloat16),
            "wo": np.ascontiguousarray(wo[hs]).reshape(HDG, D).astype(ml_dtypes.bfloat16),
            "bq": np.ascontiguousarray(bq[hs]).reshape(HDG),
            "bk": np.ascontiguousarray(bk[hs]).reshape(HDG),
            "onesv": onesv,
            })
    return in_maps


def kernel(x, wq, bq, wk, bk, wv, bv, wo, bo, _timing=None):
    wo = np.asarray(wo, dtype=np.float32)
    bv = np.asarray(bv, dtype=np.float32)
    bo = np.asarray(bo, dtype=np.float32)

    nc = _get_program()
    in_maps = make_in_maps(
        {"x": x, "wq": wq, "wk": wk, "wv": wv, "wo": wo, "bq": bq, "bk": bk})

    res = run_bass_kernel_spmd(nc, in_maps, list(range(8)))
    if _timing is not None:
        _timing["exec_time_ns"] = res.exec_time_ns
        _timing["results"] = res

    # host-side unshard: sum the two head-group partials per batch,
    # add the linear bias contributions (bo + sum_h bv_h @ wo_h).
    bias_row = bo + np.einsum("hd,hdo->o", bv, wo)
    out = np.empty((B, L, D), dtype=np.float32)
    for b in range(B):
        out[b] = res.results[2 * b]["y"] + res.results[2 * b + 1]["y"] + bias_row
    return out


# revision 25
# speedup vs baseline: 1.1594x; 1.1594x over previous
"""Multi-head dot-product attention on 8 Trainium2 NeuronCores.

Sharding: 8 cores = 4 batches x 2 head-groups (8 heads each).
Each core computes its batch's QKV projections (its 8 heads), full
softmax attention for those heads, and a partial output projection.
The host sums the two head-group partials per batch and adds the
(linear) bo / bv contributions.

Per-core pipeline (all matmuls bf16 with fp32 PSUM accumulation):
  A: Q^T,K^T [hd, L] and V [L, hd] projections from host-transposed X^T
  B: head-PAIR structure, 512-wide q chunks. All phase-B matmuls run in
     64x128 row-tiled mode: the pair's scores S^T execute concurrently on
     PE row tiles T0/T8 (head 2j on SBUF partitions 0-63, head 2j+1 on
     64-127), and each P.V matmul is split into two sequential 64-row
     half-contractions accumulating into one PSUM tile, so the PE never
     pays a tiling-mode-switch drain inside the loop.
     Exp evacuation of S^T (PSUM fp32 -> SBUF bf16 P) is split between
     ScalarE (exact exp LUT) and VectorE (Schraudolph bit-trick exp:
     one tensor_scalar mult+add with int16 round-to-nearest output whose
     bits are the bf16 of 2^(s*log2e); ~1.8% rms multiplicative noise on
     ~40% of tiles, gate impact ~9e-3) so neither engine bottlenecks.
     A ones-column appended to V yields softmax denominators in PSUM row
     64 for free; denominators are reciprocated straight out of PSUM via
     the single-pass DVE reciprocal_approx_fast, staged to DRAM, and
     partition-broadcast back; the normalization multiplies are deferred
     one qc-block so the DMA roundtrip never stalls the VectorE queue.
  C: output projection (partial - contracts this core's 8 heads)

Shapes (hardcoded): B=4, L=2048, D=1024, H=16, Hd=64.
"""

import os
import sys

for _p in ("/opt/trn_rl_repo", os.path.expanduser("~/.axon_site/_ro/trn_rl_repo")):
    if os.path.isdir(_p) and _p not in sys.path:
        sys.path.insert(0, _p)

from contextlib import ExitStack

import ml_dtypes
import numpy as np

import concourse.bass as bass
import concourse.tile as tile
from concourse import bacc, mybir
from concourse.bass_utils import run_bass_kernel_spmd

F32 = mybir.dt.float32
BF16 = mybir.dt.bfloat16
I16 = mybir.dt.int16

B, L, D, H, Hd = 4, 2048, 1024, 16, 64
HG = H // 2  # heads per core (head group)
HDG = HG * Hd  # 512: per-core projected width
KT = L // 128  # 16 k/l tiles
MT = HDG // 128  # 4 hd tiles
NQC = 4  # 512-wide q chunks
QW = L // NQC  # 512
EXP_SCALE = 1.0 / np.sqrt(Hd)
# Schraudolph exp: bf16 bits = int16(s * EXP_C1 + EXP_C0)
EXP_C1 = float(128.0 * np.log2(np.e) * EXP_SCALE)
EXP_C0 = 16248.5
# per-t: head a always evacuates via ScalarE exp; head b via VectorE
# Schraudolph except these t's (ACT:DVE = 19:13 per pair-unit)
ACT_EXTRA_T = frozenset({2, 7, 12})

# HW-validated configuration (hw_bisect.py): reciprocal_approx_fast reading
# PSUM directly returns garbage on hardware, so the denominator row is
# staged through SBUF first (T_RECIP=0). T_SCHRAU toggles the DVE
# approximate-exp path for debugging.
T_SCHRAU = os.environ.get("T_SCHRAU", "1") == "1"  # DVE Schraudolph exp
T_RECIP = os.environ.get("T_RECIP", "0") == "1"    # recip direct from PSUM


def build_program(loop_n=1, phases='ABC'):
    nc = bacc.Bacc()

    xt_d = nc.dram_tensor("xt", [D, L], BF16, kind="ExternalInput")
    wq_d = nc.dram_tensor("wq", [D, HDG], BF16, kind="ExternalInput")
    wk_d = nc.dram_tensor("wk", [D, HDG], BF16, kind="ExternalInput")
    wv_d = nc.dram_tensor("wv", [D, HDG], BF16, kind="ExternalInput")
    wo_d = nc.dram_tensor("wo", [HDG, D], BF16, kind="ExternalInput")
    bq_d = nc.dram_tensor("bq", [HDG], F32, kind="ExternalInput")
    bk_d = nc.dram_tensor("bk", [HDG], F32, kind="ExternalInput")
    onesv_d = nc.dram_tensor("onesv", [128, KT, HG], BF16, kind="ExternalInput")
    recip_d = nc.dram_tensor("recip_scratch", [HG * NQC, QW], F32)
    y_d = nc.dram_tensor("y", [L, D], F32, kind="ExternalOutput")

    with tile.TileContext(nc) as tc, \
            nc.allow_low_precision(reason="bf16/approx-exp attention internals are intentional"):
        if loop_n == 1:
            with ExitStack() as ctx:
                kernel_body(ctx, tc, xt_d, wq_d, wk_d, wv_d, wo_d, bq_d, bk_d,
                            onesv_d, recip_d, y_d, phases)
        else:
            with tc.For_i(0, loop_n, 1):
                with ExitStack() as ctx:
                    kernel_body(ctx, tc, xt_d, wq_d, wk_d, wv_d, wo_d, bq_d,
                                bk_d, onesv_d, recip_d, y_d, phases)
    nc.compile()
    return nc


def kernel_body(ctx, tc, xt_d, wq_d, wk_d, wv_d, wo_d, bq_d, bk_d,
                onesv_d, recip_d, y_d, phases="ABC"):
    nc = tc.nc
    Exp = mybir.ActivationFunctionType.Exp

    persist = ctx.enter_context(tc.tile_pool(name="persist", bufs=1))

    # persistent tensors. Q^T is stored zero-padded per head so the scores
    # matmuls can run with K=128 in the SAME 128-row PE mode as every other
    # matmul: alternating 64-row and 128-row tiling modes costs a full PE
    # drain per switch (~2.5x per-matmul, measured in micro.py). The other
    # head's K^T rows multiply the zero half, so K^T needs no padding.
    qt0_sb = persist.tile([128, MT, L], BF16)  # even head Q^T in rows 0-63, rows 64-127 zero
    qt1_sb = persist.tile([128, MT, L], BF16)  # odd head Q^T in rows 64-127, rows 0-63 zero
    kt_sb = persist.tile([128, MT, L], BF16)   # K^T
    v_sb = persist.tile([128, KT, HG * (Hd + 1)], BF16)  # V + ones col per head
    bq_sb = persist.tile([128, MT], F32)
    bk_sb = persist.tile([128, MT], F32)

    nc.sync.dma_start(bq_sb[:], bq_d.ap().rearrange("(m p) -> p m", p=128))
    nc.sync.dma_start(bk_sb[:], bk_d.ap().rearrange("(m p) -> p m", p=128))
    # zero the pad halves of the split Q^T (Pool engine; hidden under phase A)
    nc.gpsimd.memset(qt0_sb[64:128, :, :], 0)
    nc.gpsimd.memset(qt1_sb[0:64, :, :], 0)
    # ones columns of V (col Hd of each head's 65-wide block)
    vv = v_sb[:].rearrange("p t (h c) -> p t h c", h=HG)
    nc.sync.dma_start(vv[:, :, :, Hd : Hd + 1].rearrange("p t h c -> p t (h c)"),
                      onesv_d.ap())

    # ---------------- Phase A: QKV projections ----------------
    with tc.tile_pool(name="xtp", bufs=1) as xtp, \
         tc.tile_pool(name="wvp", bufs=1) as wvp, \
         tc.tile_pool(name="wst", bufs=6) as wst, \
         tc.tile_pool(name="ppa", bufs=8, space="PSUM") as ppa:
        xt_sb = xtp.tile([128, D // 128, L], BF16)
        xt_r = xt_d.ap().rearrange("(k p) n -> p k n", p=128)
        # spread input DMAs over 3 DGE queues (SP/Pool/ACT; DVE can't
        # trigger DMAs here) so phase A isn't gated on two queues' serial
        # descriptor processing
        dmae = [nc.sync, nc.gpsimd, nc.scalar]
        for k in range(D // 128):
            dmae[k % len(dmae)].dma_start(xt_sb[:, k, :], xt_r[:, k, :])

        # Q^T, K^T: out[m-tile] = W[:, m-tile].T @ X^T, chunks of 512 over l.
        # m-tile outer / tensor inner so kt[0] (which gates phase B's first
        # scores) is ready after the first m pass, not after all of Q.
        wq_r = wq_d.ap().rearrange("(k p) m -> p k m", p=128)
        wk_r = wk_d.ap().rearrange("(k p) m -> p k m", p=128)
        for m in range(MT):
            for is_q, w_r, bias_sb in ((True, wq_r, bq_sb),
                                       (False, wk_r, bk_sb)):
                psums = []
                for lc in range(4):
                    psums.append(ppa.tile([128, 512], F32, tag="pp", name=f"pp{lc}"))
                wt = wst.tile([128, D // 128, 128], BF16, tag="wt")
                dmae[m % 2].dma_start(wt[:], w_r[:, :, m * 128 : (m + 1) * 128])
                for k in range(D // 128):
                    for lc in range(4):
                        nc.tensor.matmul(
                            psums[lc][:],
                            wt[:, k, :],
                            xt_sb[:, k, lc * 512 : (lc + 1) * 512],
                            start=(k == 0),
                            stop=(k == D // 128 - 1),
                        )
                for lc in range(4):
                    sl = slice(lc * 512, (lc + 1) * 512)
                    if is_q:
                        # split evac into the per-head zero-padded tiles
                        nc.scalar.activation(
                            qt0_sb[0:64, m, sl], psums[lc][0:64, :],
                            mybir.ActivationFunctionType.Identity,
                            bias=bias_sb[0:64, m : m + 1],
                        )
                        nc.scalar.activation(
                            qt1_sb[64:128, m, sl], psums[lc][64:128, :],
                            mybir.ActivationFunctionType.Identity,
                            bias=bias_sb[64:128, m : m + 1],
                        )
                    else:
                        nc.scalar.activation(
                            kt_sb[:, m, sl], psums[lc][:],
                            mybir.ActivationFunctionType.Identity,
                            bias=bias_sb[:, m : m + 1],
                        )

        # V in natural layout: V[l-tile] = X^T[:, l-tile].T @ Wv
        wv_sb = wvp.tile([128, D // 128, HDG], BF16)
        wv_r = wv_d.ap().rearrange("(k p) n -> p k n", p=128)
        for k in range(D // 128):
            dmae[k % 2].dma_start(wv_sb[:, k, :], wv_r[:, k, :])
        for lt in range(KT):
            ps_v = ppa.tile([128, 512], F32, tag="pp")
            for k in range(D // 128):
                nc.tensor.matmul(
                    ps_v[:],
                    xt_sb[:, k, lt * 128 : (lt + 1) * 128],
                    wv_sb[:, k, :],
                    start=(k == 0),
                    stop=(k == D // 128 - 1),
                )
            nc.vector.tensor_copy(
                vv[:, lt, :, 0:Hd],
                ps_v[:].rearrange("p (h c) -> p h c", h=HG),
            )

    # ---------------- Phase B: attention ----------------
    if "B" not in phases:
        return
    with tc.tile_pool(name="otn", bufs=1) as otnp:
      otn_sb = otnp.tile([128, MT, L], BF16)  # normalized O^T
      with tc.tile_pool(name="pb", bufs=12) as pb, \
           tc.tile_pool(name="nrm", bufs=6) as nrm, \
           tc.tile_pool(name="psb", bufs=4, space="PSUM") as psb, \
           tc.tile_pool(name="pob", bufs=2, space="PSUM") as pob:

        # deferred normalization closures, flushed one qc-block later
        pending_norm = []

        def flush_norm():
            while pending_norm:
                pending_norm.pop(0)()

        def make_norm(j, qc):
            # broadcast DMAs issue NOW (the recip rows are already on their
            # way to DRAM); only the multiply is deferred one qc block
            q_sl = slice(qc * QW, (qc + 1) * QW)
            rb = nrm.tile([128, QW], F32, tag="rb")
            for hi in range(2):
                row = qc * HG + 2 * j + hi
                nc.gpsimd.dma_start(
                    rb[hi * 64 : hi * 64 + 64, :],
                    recip_d.ap()[row : row + 1, :].partition_broadcast(64))

            def run():
                sl = otn_sb[:, j, q_sl]
                nc.vector.tensor_tensor(sl, sl, rb[:],
                                        op=mybir.AluOpType.mult)
            return run

        for qc in range(NQC):
            q_sl = slice(qc * QW, (qc + 1) * QW)
            for j in range(MT):
                heads = (2 * j, 2 * j + 1)
                po = [pob.tile([65, QW], F32, tag=f"po{hi}", name=f"po{hi}")
                      for hi in range(2)]
                # PV consumes the P tile from TWO iterations back so the
                # exp evacuation has a full extra PE iteration of slack
                prev = [None, None]
                prev2 = [None, None]
                for t in range(KT + 2):
                    cur = [None, None]
                    if t < KT:
                        ps_s = [None, None]
                        # scores with K=128 via the zero-padded per-head Q^T:
                        # rows of the other head in kt_sb hit the zero half,
                        # keeping the whole loop in one PE tiling mode
                        for hi in range(2):
                            qt_z = qt0_sb if hi == 0 else qt1_sb
                            ps_s[hi] = psb.tile([128, QW], F32, tag="ps",
                                                name=f"ps{hi}")
                            nc.tensor.matmul(
                                ps_s[hi][:],
                                kt_sb[:, j, t * 128 : (t + 1) * 128],
                                qt_z[:, j, q_sl],
                                start=True,
                                stop=True,
                            )
                        for hi in range(2):
                            pt = pb.tile([128, QW], BF16, tag="pt", name="pt")
                            if not T_SCHRAU or hi == 0 or t in ACT_EXTRA_T:
                                nc.scalar.activation(pt[:], ps_s[hi][:], Exp,
                                                     scale=EXP_SCALE)
                            else:
                                nc.vector.tensor_scalar(
                                    pt[:].bitcast(I16), ps_s[hi][:],
                                    EXP_C1, EXP_C0,
                                    op0=mybir.AluOpType.mult,
                                    op1=mybir.AluOpType.add,
                                )
                            cur[hi] = pt
                    if t > 1:
                        h0, h1 = heads
                        c0 = h0 * (Hd + 1)
                        c1 = h1 * (Hd + 1)
                        for hi in range(2):
                            cc = c0 if hi == 0 else c1
                            nc.tensor.matmul(
                                po[hi][:],
                                v_sb[:, t - 2, cc : cc + Hd + 1],
                                prev2[hi][:],
                                start=(t == 2),
                                stop=(t == KT + 1),
                            )
                    prev2 = prev
                    prev = cur
                # unit end: evacuate unnormalized O^T rows + reciprocals
                for hi in range(2):
                    h = heads[hi]
                    r = hi * 64
                    row = qc * HG + h
                    nc.vector.tensor_copy(otn_sb[r : r + 64, j, q_sl],
                                          po[hi][0:64, :])
                    # DVE single-partition writes must start at partition 0:
                    # stage the reciprocal row in a rotating tile, then DMA
                    rrow = nrm.tile([128, QW], F32, tag="rr", bufs=3)
                    if T_RECIP:
                        nc.vector.reciprocal_approx_fast(
                            rrow[0:1, :], po[hi][64:65, :])
                    else:
                        rr2 = nrm.tile([128, QW], F32, tag="rr2", bufs=3)
                        nc.vector.tensor_copy(rr2[0:1, :], po[hi][64:65, :])
                        nc.vector.reciprocal_approx_fast(
                            rrow[0:1, :], rr2[0:1, :])
                    nc.sync.dma_start(recip_d.ap()[row : row + 1, :],
                                      rrow[0:1, :])
                # run one deferred norm from the previous qc block BEFORE
                # allocating this unit's rb tile (so a freed rb slot's last
                # reader is always emitted before its next writer); by now
                # its recip DMA roundtrip has long completed, so the DVE
                # queue never stalls on it
                while len(pending_norm) >= MT + 1:
                    pending_norm.pop(0)()
                pending_norm.append(make_norm(j, qc))
        flush_norm()

      # ---------------- Phase C: output projection ----------------
      if "C" not in phases:
          return
      with tc.tile_pool(name="wop", bufs=1) as wop, \
           tc.tile_pool(name="yst", bufs=4) as yst, \
           tc.tile_pool(name="pyc", bufs=4, space="PSUM") as pyc:
          wo_sb = wop.tile([128, MT, D], BF16)
          nc.sync.dma_start(wo_sb[:], wo_d.ap().rearrange("(k p) n -> p k n", p=128))
          for mq in range(KT):
              for nch in range(2):
                  n_sl = slice(nch * 512, (nch + 1) * 512)
                  ps_y = pyc.tile([128, 512], F32, tag="py")
                  for k in range(MT):
                      nc.tensor.matmul(
                          ps_y[:],
                          otn_sb[:, k, mq * 128 : (mq + 1) * 128],
                          wo_sb[:, k, n_sl],
                          start=(k == 0),
                          stop=(k == MT - 1),
                      )
                  yt = yst.tile([128, 512], F32, tag="yt")
                  if (mq + nch) % 2 == 0:
                      nc.vector.tensor_copy(yt[:], ps_y[:])
                  else:
                      nc.scalar.activation(
                          yt[:], ps_y[:],
                          mybir.ActivationFunctionType.Identity)
                  nc.sync.dma_start(
                      y_d.ap()[mq * 128 : (mq + 1) * 128, n_sl], yt[:])


_PROGRAM_CACHE = {}


def _get_program():
    if "nc" not in _PROGRAM_CACHE:
        _PROGRAM_CACHE["nc"] = build_program()
    return _PROGRAM_CACHE["nc"]


def make_in_maps(inputs):
    x = np.asarray(inputs["x"], dtype=np.float32)
    wq = np.asarray(inputs["wq"], dtype=np.float32)
    wk = np.asarray(inputs["wk"], dtype=np.float32)
    wv = np.asarray(inputs["wv"], dtype=np.float32)
    wo = np.asarray(inputs["wo"], dtype=np.float32)
    bq = np.asarray(inputs["bq"], dtype=np.float32)
    bk = np.asarray(inputs["bk"], dtype=np.float32)

    onesv = np.ones((128, KT, HG), dtype=ml_dtypes.bfloat16)

    in_maps = []
    for c in range(8):
        b, g = divmod(c, 2)
        hs = slice(g * HG, (g + 1) * HG)
        in_maps.append({
            "xt": np.ascontiguousarray(x[b].T).astype(ml_dtypes.bfloat16),
            "wq": np.ascontiguousarray(wq[:, hs, :]).reshape(D, HDG).astype(ml_dtypes.bfloat16),
            "wk": np.ascontiguousarray(wk[:, hs, :]).reshape(D, HDG).astype(ml_dtypes.bfloat16),
            "wv": np.ascontiguousarray(wv[:, hs, :]).reshape(D, HDG).astype(ml_dtypes.bfloat16),
            "wo": np.ascontiguousarray(wo[hs]).reshape(HDG, D).astype(ml_dtypes.bfloat16),
            "bq": np.ascontiguousarray(bq[hs]).reshape(HDG),
            "bk": np.ascontiguousarray(bk[hs]).reshape(HDG),
            "onesv": onesv,
            })
    return in_maps


def kernel(x, wq, bq, wk, bk, wv, bv, wo, bo, _timing=None):
    wo = np.asarray(wo, dtype=np.float32)
    bv = np.asarray(bv, dtype=np.float32)
    bo = np.asarray(bo, dtype=np.float32)

    nc = _get_program()
    in_maps = make_in_maps(
        {"x": x, "wq": wq, "wk": wk, "wv": wv, "wo": wo, "bq": bq, "bk": bk})

    res = run_bass_kernel_spmd(nc, in_maps, list(range(8)))
    if _timing is not None:
        _timing["exec_time_ns"] = res.exec_time_ns
        _timing["results"] = res

    # host-side unshard: sum the two head-group partials per batch,
    # add the linear bias contributions (bo + sum_h bv_h @ wo_h).
    bias_row = bo + np.einsum("hd,hdo->o", bv, wo)
    out = np.empty((B, L, D), dtype=np.float32)
    for b in range(B):
        out[b] = res.results[2 * b]["y"] + res.results[2 * b + 1]["y"] + bias_row
    return out


# revision 38
# speedup vs baseline: 1.1883x; 1.0249x over previous
"""Multi-head dot-product attention on 8 Trainium2 NeuronCores.

Sharding: 8 cores = 4 batches x 2 head-groups (8 heads each).
Each core computes its batch's QKV projections (its 8 heads), full
softmax attention for those heads, and a partial output projection.
The host sums the two head-group partials per batch and adds the
(linear) bo / bv contributions.

Per-core pipeline (all matmuls bf16 with fp32 PSUM accumulation):
  A: Q^T,K^T [hd, L] and V [L, hd] projections from host-transposed X^T
  B: head-PAIR structure, 512-wide q chunks. All phase-B matmuls run in
     64x128 row-tiled mode: the pair's scores S^T execute concurrently on
     PE row tiles T0/T8 (head 2j on SBUF partitions 0-63, head 2j+1 on
     64-127), and each P.V matmul is split into two sequential 64-row
     half-contractions accumulating into one PSUM tile, so the PE never
     pays a tiling-mode-switch drain inside the loop.
     Exp evacuation of S^T (PSUM fp32 -> SBUF bf16 P) is split between
     ScalarE (exact exp LUT) and VectorE (Schraudolph bit-trick exp:
     one tensor_scalar mult+add with int16 round-to-nearest output whose
     bits are the bf16 of 2^(s*log2e); ~1.8% rms multiplicative noise on
     ~40% of tiles, gate impact ~9e-3) so neither engine bottlenecks.
     A ones-column appended to V yields softmax denominators in PSUM row
     64 for free; denominators are reciprocated straight out of PSUM via
     the single-pass DVE reciprocal_approx_fast, staged to DRAM, and
     partition-broadcast back; the normalization multiplies are deferred
     one qc-block so the DMA roundtrip never stalls the VectorE queue.
  C: output projection (partial - contracts this core's 8 heads)

Shapes (hardcoded): B=4, L=2048, D=1024, H=16, Hd=64.
"""

import os
import sys

for _p in ("/opt/trn_rl_repo", os.path.expanduser("~/.axon_site/_ro/trn_rl_repo")):
    if os.path.isdir(_p) and _p not in sys.path:
        sys.path.insert(0, _p)

from contextlib import ExitStack

import ml_dtypes
import numpy as np

import concourse.bass as bass
import concourse.tile as tile
from concourse import bacc, mybir
from concourse.bass_utils import run_bass_kernel_spmd

F32 = mybir.dt.float32
BF16 = mybir.dt.bfloat16
I16 = mybir.dt.int16

B, L, D, H, Hd = 4, 2048, 1024, 16, 64
HG = H // 2  # heads per core (head group)
HDG = HG * Hd  # 512: per-core projected width
KT = L // 128  # 16 k/l tiles
MT = HDG // 128  # 4 hd tiles
NQC = 4  # 512-wide q chunks
QW = L // NQC  # 512
EXP_SCALE = 1.0 / np.sqrt(Hd)
# Schraudolph exp: bf16 bits = int16(s * EXP_C1 + EXP_C0)
EXP_C1 = float(128.0 * np.log2(np.e) * EXP_SCALE)
EXP_C0 = 16248.5
# per-t: head a always evacuates via ScalarE exp; head b via VectorE
# Schraudolph except these t's (ACT:DVE = 19:13 per pair-unit)
ACT_EXTRA_T = frozenset({2, 7, 12})

# HW-validated configuration (hw_bisect.py): reciprocal_approx_fast reading
# PSUM directly returns garbage on hardware, so the denominator row is
# staged through SBUF first (T_RECIP=0). T_SCHRAU toggles the DVE
# approximate-exp path for debugging.
T_SCHRAU = os.environ.get("T_SCHRAU", "1") == "1"  # DVE Schraudolph exp
T_RECIP = os.environ.get("T_RECIP", "0") == "1"    # recip direct from PSUM


def build_program(loop_n=1, phases='ABC'):
    nc = bacc.Bacc()

    xt_d = nc.dram_tensor("xt", [D, L], BF16, kind="ExternalInput")
    wq_d = nc.dram_tensor("wq", [D, HDG], BF16, kind="ExternalInput")
    wk_d = nc.dram_tensor("wk", [D, HDG], BF16, kind="ExternalInput")
    wv_d = nc.dram_tensor("wv", [D, HDG], BF16, kind="ExternalInput")
    wo_d = nc.dram_tensor("wo", [HDG, D], BF16, kind="ExternalInput")
    bq_d = nc.dram_tensor("bq", [HDG], F32, kind="ExternalInput")
    bk_d = nc.dram_tensor("bk", [HDG], F32, kind="ExternalInput")
    onesv_d = nc.dram_tensor("onesv", [128, KT, HG], BF16, kind="ExternalInput")
    recip_d = nc.dram_tensor("recip_scratch", [HG * NQC, QW], F32)
    y_d = nc.dram_tensor("y", [L, D], F32, kind="ExternalOutput")

    with tile.TileContext(nc) as tc, \
            nc.allow_low_precision(reason="bf16/approx-exp attention internals are intentional"):
        if loop_n == 1:
            with ExitStack() as ctx:
                kernel_body(ctx, tc, xt_d, wq_d, wk_d, wv_d, wo_d, bq_d, bk_d,
                            onesv_d, recip_d, y_d, phases)
        else:
            with tc.For_i(0, loop_n, 1):
                with ExitStack() as ctx:
                    kernel_body(ctx, tc, xt_d, wq_d, wk_d, wv_d, wo_d, bq_d,
                                bk_d, onesv_d, recip_d, y_d, phases)
    nc.compile()
    return nc


def kernel_body(ctx, tc, xt_d, wq_d, wk_d, wv_d, wo_d, bq_d, bk_d,
                onesv_d, recip_d, y_d, phases="ABC"):
    nc = tc.nc
    Exp = mybir.ActivationFunctionType.Exp

    persist = ctx.enter_context(tc.tile_pool(name="persist", bufs=1))

    # persistent tensors. Q^T is stored zero-padded per head so the scores
    # matmuls can run with K=128 in the SAME 128-row PE mode as every other
    # matmul: alternating 64-row and 128-row tiling modes costs a full PE
    # drain per switch (~2.5x per-matmul, measured in micro.py). The other
    # head's K^T rows multiply the zero half, so K^T needs no padding.
    qt0_sb = persist.tile([128, MT, L], BF16)  # even head Q^T in rows 0-63, rows 64-127 zero
    qt1_sb = persist.tile([128, MT, L], BF16)  # odd head Q^T in rows 64-127, rows 0-63 zero
    kt_sb = persist.tile([128, MT, L], BF16)   # K^T
    v_sb = persist.tile([128, KT, HG * (Hd + 1)], BF16)  # V + ones col per head
    bq_sb = persist.tile([128, MT], F32)
    bk_sb = persist.tile([128, MT], F32)

    nc.sync.dma_start(bq_sb[:], bq_d.ap().rearrange("(m p) -> p m", p=128))
    nc.sync.dma_start(bk_sb[:], bk_d.ap().rearrange("(m p) -> p m", p=128))
    # zero the pad halves of the split Q^T (Pool engine; hidden under phase A)
    nc.gpsimd.memset(qt0_sb[64:128, :, :], 0)
    nc.gpsimd.memset(qt1_sb[0:64, :, :], 0)
    # ones columns of V (col Hd of each head's 65-wide block)
    vv = v_sb[:].rearrange("p t (h c) -> p t h c", h=HG)
    nc.sync.dma_start(vv[:, :, :, Hd : Hd + 1].rearrange("p t h c -> p t (h c)"),
                      onesv_d.ap())

    # ---------------- Phase A: QKV projections ----------------
    with tc.tile_pool(name="xtp", bufs=1) as xtp, \
         tc.tile_pool(name="wvp", bufs=1) as wvp, \
         tc.tile_pool(name="wst", bufs=6) as wst, \
         tc.tile_pool(name="ppa", bufs=8, space="PSUM") as ppa:
        xt_sb = xtp.tile([128, D // 128, L], BF16)
        xt_r = xt_d.ap().rearrange("(k p) n -> p k n", p=128)
        # spread input DMAs over 3 DGE queues (SP/Pool/ACT; DVE can't
        # trigger DMAs here), split into half-chunks for more DMA-engine
        # parallelism, so phase A isn't gated on serial descriptor work
        dmae = [nc.sync, nc.gpsimd, nc.scalar]
        di = 0
        for k in range(D // 128):
            for half in range(2):
                sl = slice(half * (L // 2), (half + 1) * (L // 2))
                dmae[di % len(dmae)].dma_start(xt_sb[:, k, sl], xt_r[:, k, sl])
                di += 1

        # Q^T, K^T: out[m-tile] = W[:, m-tile].T @ X^T, chunks of 512 over l.
        # m-tile outer / tensor inner so kt[0] (which gates phase B's first
        # scores) is ready after the first m pass, not after all of Q.
        wq_r = wq_d.ap().rearrange("(k p) m -> p k m", p=128)
        wk_r = wk_d.ap().rearrange("(k p) m -> p k m", p=128)
        for m in range(MT):
            for is_q, w_r, bias_sb in ((True, wq_r, bq_sb),
                                       (False, wk_r, bk_sb)):
                psums = []
                for lc in range(4):
                    psums.append(ppa.tile([128, 512], F32, tag="pp", name=f"pp{lc}"))
                wt = wst.tile([128, D // 128, 128], BF16, tag="wt")
                dmae[m % 2].dma_start(wt[:], w_r[:, :, m * 128 : (m + 1) * 128])
                for k in range(D // 128):
                    for lc in range(4):
                        nc.tensor.matmul(
                            psums[lc][:],
                            wt[:, k, :],
                            xt_sb[:, k, lc * 512 : (lc + 1) * 512],
                            start=(k == 0),
                            stop=(k == D // 128 - 1),
                        )
                for lc in range(4):
                    sl = slice(lc * 512, (lc + 1) * 512)
                    if is_q:
                        # split evac into the per-head zero-padded tiles
                        nc.scalar.activation(
                            qt0_sb[0:64, m, sl], psums[lc][0:64, :],
                            mybir.ActivationFunctionType.Identity,
                            bias=bias_sb[0:64, m : m + 1],
                        )
                        nc.scalar.activation(
                            qt1_sb[64:128, m, sl], psums[lc][64:128, :],
                            mybir.ActivationFunctionType.Identity,
                            bias=bias_sb[64:128, m : m + 1],
                        )
                    else:
                        nc.scalar.activation(
                            kt_sb[:, m, sl], psums[lc][:],
                            mybir.ActivationFunctionType.Identity,
                            bias=bias_sb[:, m : m + 1],
                        )

        # V in natural layout: V[l-tile] = X^T[:, l-tile].T @ Wv
        wv_sb = wvp.tile([128, D // 128, HDG], BF16)
        wv_r = wv_d.ap().rearrange("(k p) n -> p k n", p=128)
        for k in range(D // 128):
            dmae[k % 2].dma_start(wv_sb[:, k, :], wv_r[:, k, :])
        for lt in range(KT):
            ps_v = ppa.tile([128, 512], F32, tag="pp")
            for k in range(D // 128):
                nc.tensor.matmul(
                    ps_v[:],
                    xt_sb[:, k, lt * 128 : (lt + 1) * 128],
                    wv_sb[:, k, :],
                    start=(k == 0),
                    stop=(k == D // 128 - 1),
                )
            nc.vector.tensor_copy(
                vv[:, lt, :, 0:Hd],
                ps_v[:].rearrange("p (h c) -> p h c", h=HG),
            )

    # ---------------- Phase B: attention ----------------
    if "B" not in phases:
        return
    with tc.tile_pool(name="otn", bufs=1) as otnp:
      otn_sb = otnp.tile([128, MT, L], BF16)  # normalized O^T
      with tc.tile_pool(name="pb", bufs=12) as pb, \
           tc.tile_pool(name="nrm", bufs=6) as nrm, \
           tc.tile_pool(name="psb", bufs=4, space="PSUM") as psb, \
           tc.tile_pool(name="pob", bufs=2, space="PSUM") as pob:

        # phase C's 1MB wo load is emitted HERE (phase B start): the sync
        # queue is idle now, so it transfers during early B. Emitted at C it
        # stalls C's first matmuls ~40us (SP only reaches it after B's last
        # sem-gated reciprocal DMA); emitted at body start it delays the xt
        # chunks gating DMA-bound phase A (measured 40us WORSE).
        wo_sb = otnp.tile([128, MT, D], BF16)
        nc.sync.dma_start(wo_sb[:], wo_d.ap().rearrange("(k p) n -> p k n", p=128))

        # deferred normalization closures, flushed one qc-block later
        pending_norm = []

        def flush_norm():
            while pending_norm:
                pending_norm.pop(0)()

        def make_norm(j, qc):
            # broadcast DMAs issue NOW (the recip rows are already on their
            # way to DRAM); only the multiply is deferred one qc block
            q_sl = slice(qc * QW, (qc + 1) * QW)
            rb = nrm.tile([128, QW], F32, tag="rb")
            for hi in range(2):
                row = qc * HG + 2 * j + hi
                nc.gpsimd.dma_start(
                    rb[hi * 64 : hi * 64 + 64, :],
                    recip_d.ap()[row : row + 1, :].partition_broadcast(64))

            def run():
                sl = otn_sb[:, j, q_sl]
                nc.vector.tensor_tensor(sl, sl, rb[:],
                                        op=mybir.AluOpType.mult)
            return run

        for qc in range(NQC):
            q_sl = slice(qc * QW, (qc + 1) * QW)
            for j in range(MT):
                heads = (2 * j, 2 * j + 1)
                po = [pob.tile([65, QW], F32, tag=f"po{hi}", name=f"po{hi}")
                      for hi in range(2)]
                # PV consumes the P tile from TWO iterations back so the
                # exp evacuation has a full extra PE iteration of slack
                # (depth 3 measured slightly worse, depth 1 ~10us worse)
                prev = [None, None]
                prev2 = [None, None]
                for t in range(KT + 2):
                    cur = [None, None]
                    if t < KT:
                        ps_s = [None, None]
                        # scores with K=128 via the zero-padded per-head Q^T:
                        # rows of the other head in kt_sb hit the zero half,
                        # keeping the whole loop in one PE tiling mode
                        for hi in range(2):
                            qt_z = qt0_sb if hi == 0 else qt1_sb
                            ps_s[hi] = psb.tile([128, QW], F32, tag="ps",
                                                name=f"ps{hi}")
                            nc.tensor.matmul(
                                ps_s[hi][:],
                                kt_sb[:, j, t * 128 : (t + 1) * 128],
                                qt_z[:, j, q_sl],
                                start=True,
                                stop=True,
                            )
                        for hi in range(2):
                            pt = pb.tile([128, QW], BF16, tag="pt", name="pt")
                            if not T_SCHRAU or hi == 0 or t in ACT_EXTRA_T:
                                nc.scalar.activation(pt[:], ps_s[hi][:], Exp,
                                                     scale=EXP_SCALE)
                            else:
                                nc.vector.tensor_scalar(
                                    pt[:].bitcast(I16), ps_s[hi][:],
                                    EXP_C1, EXP_C0,
                                    op0=mybir.AluOpType.mult,
                                    op1=mybir.AluOpType.add,
                                )
                            cur[hi] = pt
                    if t > 1:
                        h0, h1 = heads
                        c0 = h0 * (Hd + 1)
                        c1 = h1 * (Hd + 1)
                        for hi in range(2):
                            cc = c0 if hi == 0 else c1
                            nc.tensor.matmul(
                                po[hi][:],
                                v_sb[:, t - 2, cc : cc + Hd + 1],
                                prev2[hi][:],
                                start=(t == 2),
                                stop=(t == KT + 1),
                            )
                    prev2 = prev
                    prev = cur
                # unit end: evacuate unnormalized O^T rows + reciprocals
                for hi in range(2):
                    h = heads[hi]
                    r = hi * 64
                    row = qc * HG + h
                    # O^T evac on ScalarE: DVE (13 exps + recip chain) is
                    # busier than ACT (19 exps) per unit; Identity lives in
                    # the same ACT table set as Exp so no table reload
                    nc.scalar.activation(
                        otn_sb[r : r + 64, j, q_sl], po[hi][0:64, :],
                        mybir.ActivationFunctionType.Identity)
                    # DVE single-partition writes must start at partition 0:
                    # stage the reciprocal row in a rotating tile, then DMA
                    rrow = nrm.tile([128, QW], F32, tag="rr", bufs=3)
                    if T_RECIP:
                        nc.vector.reciprocal_approx_fast(
                            rrow[0:1, :], po[hi][64:65, :])
                    else:
                        rr2 = nrm.tile([128, QW], F32, tag="rr2", bufs=3)
                        nc.vector.tensor_copy(rr2[0:1, :], po[hi][64:65, :])
                        nc.vector.reciprocal_approx_fast(
                            rrow[0:1, :], rr2[0:1, :])
                    nc.sync.dma_start(recip_d.ap()[row : row + 1, :],
                                      rrow[0:1, :])
                # run one deferred norm from the previous qc block BEFORE
                # allocating this unit's rb tile (so a freed rb slot's last
                # reader is always emitted before its next writer); by now
                # its recip DMA roundtrip has long completed, so the DVE
                # queue never stalls on it
                while len(pending_norm) >= MT + 1:
                    pending_norm.pop(0)()
                pending_norm.append(make_norm(j, qc))
        flush_norm()

      # ---------------- Phase C: output projection ----------------
      if "C" not in phases:
          return
      with tc.tile_pool(name="yst", bufs=4) as yst, \
           tc.tile_pool(name="pyc", bufs=4, space="PSUM") as pyc:
          for mq in range(KT):
              for nch in range(2):
                  n_sl = slice(nch * 512, (nch + 1) * 512)
                  ps_y = pyc.tile([128, 512], F32, tag="py")
                  for k in range(MT):
                      nc.tensor.matmul(
                          ps_y[:],
                          otn_sb[:, k, mq * 128 : (mq + 1) * 128],
                          wo_sb[:, k, n_sl],
                          start=(k == 0),
                          stop=(k == MT - 1),
                      )
                  yt = yst.tile([128, 512], F32, tag="yt")
                  if (mq + nch) % 2 == 0:
                      nc.vector.tensor_copy(yt[:], ps_y[:])
                  else:
                      nc.scalar.activation(
                          yt[:], ps_y[:],
                          mybir.ActivationFunctionType.Identity)
                  nc.sync.dma_start(
                      y_d.ap()[mq * 128 : (mq + 1) * 128, n_sl], yt[:])


_PROGRAM_CACHE = {}


def _get_program():
    if "nc" not in _PROGRAM_CACHE:
        _PROGRAM_CACHE["nc"] = build_program()
    return _PROGRAM_CACHE["nc"]


def make_in_maps(inputs):
    x = np.asarray(inputs["x"], dtype=np.float32)
    wq = np.asarray(inputs["wq"], dtype=np.float32)
    wk = np.asarray(inputs["wk"], dtype=np.float32)
    wv = np.asarray(inputs["wv"], dtype=np.float32)
    wo = np.asarray(inputs["wo"], dtype=np.float32)
    bq = np.asarray(inputs["bq"], dtype=np.float32)
    bk = np.asarray(inputs["bk"], dtype=np.float32)

    onesv = np.ones((128, KT, HG), dtype=ml_dtypes.bfloat16)

    in_maps = []
    for c in range(8):
        b, g = divmod(c, 2)
        hs = slice(g * HG, (g + 1) * HG)
        in_maps.append({
            "xt": np.ascontiguousarray(x[b].T).astype(ml_dtypes.bfloat16),
            "wq": np.ascontiguousarray(wq[:, hs, :]).reshape(D, HDG).astype(ml_dtypes.bfloat16),
            "wk": np.ascontiguousarray(wk[:, hs, :]).reshape(D, HDG).astype(ml_dtypes.bfloat16),
            "wv": np.ascontiguousarray(wv[:, hs, :]).reshape(D, HDG).astype(ml_dtypes.bfloat16),
            "wo": np.ascontiguousarray(wo[hs]).reshape(HDG, D).astype(ml_dtypes.bfloat16),
            "bq": np.ascontiguousarray(bq[hs]).reshape(HDG),
            "bk": np.ascontiguousarray(bk[hs]).reshape(HDG),
            "onesv": onesv,
            })
    return in_maps


def kernel(x, wq, bq, wk, bk, wv, bv, wo, bo, _timing=None):
    wo = np.asarray(wo, dtype=np.float32)
    bv = np.asarray(bv, dtype=np.float32)
    bo = np.asarray(bo, dtype=np.float32)

    nc = _get_program()
    in_maps = make_in_maps(
        {"x": x, "wq": wq, "wk": wk, "wv": wv, "wo": wo, "bq": bq, "bk": bk})

    res = run_bass_kernel_spmd(nc, in_maps, list(range(8)))
    if _timing is not None:
        _timing["exec_time_ns"] = res.exec_time_ns
        _timing["results"] = res

    # host-side unshard: sum the two head-group partials per batch,
    # add the linear bias contributions (bo + sum_h bv_h @ wo_h).
    bias_row = bo + np.einsum("hd,hdo->o", bv, wo)
    out = np.empty((B, L, D), dtype=np.float32)
    for b in range(B):
        out[b] = res.results[2 * b]["y"] + res.results[2 * b + 1]["y"] + bias_row
    return out
